# revision 5
# baseline (speedup 1.0000x reference)
"""Trainium2 Bass kernel for the DiscourseAct GNN message-passing problem.

Computation (reference):
    x   = sent_encoding[0]                       # [SEQ, 256] f32
    du  = concat(x[1:], x[parents[1:]], axis=1)  # [SEQ-1, 512]
    pre = tanh(du @ W1.T + b1)                   # [SEQ-1, 256]
    out = pre @ W2.T + b2                        # [SEQ-1, 43]

Strategy (8 cores, data-parallel over rows, locality-aware index
partitioning):
  * Output rows are sharded contiguously: core c owns rows
    [c*32768, (c+1)*32768) (the final slot of core 7 is a dummy).
  * Host-side *index-only* preprocessing: each core's rows are sorted by
    parent index and split into 8 windows of 32768 rows so that the
    on-device `dma_gather` (int16 indices) can address them; each window
    segment is padded to a fixed 4352 slots (max real count for this data
    is ~4300; padding gathers row 0 of the window, output discarded).
  * Device: per window, bf16 transpose-mode dma_gathers (<=896 indices per
    call, the HW limit is <1024) fetch the parent rows (from the full
    replicated x) and the own rows (from the core-local shard) directly
    into feature-major SBUF layout (zero on-chip transposes), then a bf16
    matmul pipeline:
      L1: h.T[256, N] accumulated in PSUM over K=512,
      tanh+b1 on the scalar engine (PSUM -> SBUF, bf16),
      L2: out[128, 43] with h.T slices as the stationary operand,
      +b2 on the vector engine, contiguous DMA to DRAM.
  * Host reassembles (un-permutes) the rows into the full output.
"""

import numpy as np
import ml_dtypes

import concourse.bass as bass
import concourse.tile as tile
from concourse import bacc, mybir
from concourse.bass_utils import run_bass_kernel_spmd

BF16 = ml_dtypes.bfloat16

SEQ = 262144
H2 = 256            # feature dim of x
OUT_DIM = 43
N_CORES = 8
RPC = SEQ // N_CORES          # rows (output slots) per core = 32768
N_WIN = 8
WIN = SEQ // N_WIN            # rows per gather window = 32768
SPW = 4352                    # padded slots per window (34 * 128)
SLOTS = N_WIN * SPW           # 34816 compute slots per core
CALLS = (896, 896, 896, 896, 768)   # gather call sizes per window (sum=SPW)
OFFS = (0, 896, 1792, 2688, 3584)
NCALL = len(CALLS)
IDXW = CALLS[0] // 16         # idx tile cols (896/16 = 56)
CHUNK = 512                   # L1 moving-dim chunk (psum bank = 512 f32)


def _build_program():
    nc = bacc.Bacc("TRN2", target_bir_lowering=False, debug=False,
                   num_devices=N_CORES)
    dt = mybir.dt

    xbf_d = nc.dram_tensor("xbf", [SEQ, H2], dt.bfloat16, kind="ExternalInput")
    xloc_d = nc.dram_tensor("xloc", [RPC, H2], dt.bfloat16, kind="ExternalInput")
    pidx_d = nc.dram_tensor("pidx", [N_WIN * NCALL, 128, IDXW], dt.int16,
                            kind="ExternalInput")
    iidx_d = nc.dram_tensor("iidx", [N_WIN * NCALL, 128, IDXW], dt.int16,
                            kind="ExternalInput")
    w1t_d = nc.dram_tensor("w1t", [2 * H2, H2], dt.bfloat16, kind="ExternalInput")
    w2t_d = nc.dram_tensor("w2t", [H2, OUT_DIM], dt.bfloat16, kind="ExternalInput")
    b1_d = nc.dram_tensor("b1c", [H2, 1], dt.float32, kind="ExternalInput")
    b2r_d = nc.dram_tensor("b2r", [128, OUT_DIM], dt.float32, kind="ExternalInput")
    out_d = nc.dram_tensor("out", [SLOTS, OUT_DIM], dt.float32,
                           kind="ExternalOutput")

    with tile.TileContext(nc) as tc:
        with (
            tc.tile_pool(name="const", bufs=1) as const,
            tc.tile_pool(name="gslab", bufs=3) as gpool,
            tc.tile_pool(name="ht", bufs=3) as hpool,
            tc.tile_pool(name="osb", bufs=4) as opool,
            tc.tile_pool(name="hps", bufs=2, space="PSUM") as psum_h,
            tc.tile_pool(name="ops", bufs=4, space="PSUM") as psum_o,
        ):
            # --- one-time loads -------------------------------------------
            w1t_sb = []
            for k in range(4):
                t = const.tile([128, H2], dt.bfloat16, tag=f"w1t{k}",
                               name=f"w1t_sb{k}")
                nc.sync.dma_start(t[:], w1t_d.ap()[k * 128:(k + 1) * 128, :])
                w1t_sb.append(t)
            w2t_sb = []
            for k in range(2):
                t = const.tile([128, OUT_DIM], dt.bfloat16, tag=f"w2t{k}",
                               name=f"w2t_sb{k}")
                nc.sync.dma_start(t[:], w2t_d.ap()[k * 128:(k + 1) * 128, :])
                w2t_sb.append(t)
            b1_sb = []
            for m in range(2):
                t = const.tile([128, 1], dt.float32, tag=f"b1{m}",
                               name=f"b1_sb{m}")
                nc.sync.dma_start(t[:], b1_d.ap()[m * 128:(m + 1) * 128, :])
                b1_sb.append(t)
            b2r_sb = const.tile([128, OUT_DIM], dt.float32, tag="b2r")
            nc.sync.dma_start(b2r_sb[:], b2r_d.ap())
            pidx_sb, iidx_sb = [], []
            for ci in range(N_WIN * NCALL):
                tp = const.tile([128, IDXW], dt.int16, tag=f"pidx{ci}",
                                name=f"pidx_sb{ci}")
                nc.sync.dma_start(tp[:], pidx_d.ap()[ci])
                pidx_sb.append(tp)
                ti = const.tile([128, IDXW], dt.int16, tag=f"iidx{ci}",
                                name=f"iidx_sb{ci}")
                nc.sync.dma_start(ti[:], iidx_d.ap()[ci])
                iidx_sb.append(ti)

            # --- main loop ------------------------------------------------
            for w in range(N_WIN):
                for j, (nidx, off) in enumerate(zip(CALLS, OFFS)):
                    ci = w * NCALL + j
                    iw = nidx // 16
                    gp = gpool.tile([128, 2, nidx], dt.bfloat16,
                                    tag=f"gp{nidx}", name=f"gp_{ci}")
                    gi = gpool.tile([128, 2, nidx], dt.bfloat16,
                                    tag=f"gi{nidx}", name=f"gi_{ci}")
                    nc.gpsimd.dma_gather(
                        gp[:], xbf_d.ap()[w * WIN:(w + 1) * WIN, :],
                        pidx_sb[ci][:, 0:iw],
                        num_idxs=nidx, num_idxs_reg=nidx, elem_size=H2,
                        transpose=True, queue_num=0,
                    )
                    nc.gpsimd.dma_gather(
                        gi[:], xloc_d.ap()[:, :], iidx_sb[ci][:, 0:iw],
                        num_idxs=nidx, num_idxs_reg=nidx, elem_size=H2,
                        transpose=True, queue_num=0,
                    )
                    for s0 in range(0, nidx, CHUNK):
                        L = min(CHUNK, nidx - s0)
                        hps = [psum_h.tile([128, L], dt.float32, tag=f"h{m}",
                                           name=f"hps{m}")
                               for m in range(2)]
                        for m in range(2):
                            msl = slice(m * 128, (m + 1) * 128)
                            nc.tensor.matmul(hps[m][:], lhsT=w1t_sb[0][:, msl],
                                             rhs=gi[:, 0, s0:s0 + L],
                                             start=True, stop=False)
                            nc.tensor.matmul(hps[m][:], lhsT=w1t_sb[1][:, msl],
                                             rhs=gi[:, 1, s0:s0 + L],
                                             start=False, stop=False)
                            nc.tensor.matmul(hps[m][:], lhsT=w1t_sb[2][:, msl],
                                             rhs=gp[:, 0, s0:s0 + L],
                                             start=False, stop=False)
                            nc.tensor.matmul(hps[m][:], lhsT=w1t_sb[3][:, msl],
                                             rhs=gp[:, 1, s0:s0 + L],
                                             start=False, stop=True)
                        hts = []
                        for m in range(2):
                            ht = hpool.tile([128, L], dt.bfloat16, tag=f"ht{m}",
                                            name=f"ht{m}")
                            nc.scalar.activation(
                                ht[:], hps[m][:],
                                mybir.ActivationFunctionType.Tanh,
                                bias=b1_sb[m][:], scale=1.0)
                            hts.append(ht)
                        for r in range(0, L, 128):
                            ops = psum_o.tile([128, OUT_DIM], dt.float32,
                                              tag="o", name="ops")
                            nc.tensor.matmul(ops[:], lhsT=hts[0][:, r:r + 128],
                                             rhs=w2t_sb[0][:],
                                             start=True, stop=False)
                            nc.tensor.matmul(ops[:], lhsT=hts[1][:, r:r + 128],
                                             rhs=w2t_sb[1][:],
                                             start=False, stop=True)
                            osb = opool.tile([128, OUT_DIM], dt.float32,
                                             tag="osb", name="osb")
                            nc.vector.tensor_add(osb[:], ops[:], b2r_sb[:])
                            row0 = w * SPW + off + s0 + r
                            nc.sync.dma_start(out_d.ap()[row0:row0 + 128, :],
                                              osb[:])

    nc.compile()
    return nc


_CACHE = {}


def _get_program():
    if "nc" not in _CACHE:
        _CACHE["nc"] = _build_program()
    return _CACHE["nc"]


def _wrap_idx(a):
    """[SPW] int16 -> [NCALL, 128, IDXW] per-call wrapped layout:
    out[j, p, s] = call_j[s*16 + (p % 16)], zero-padded to IDXW cols."""
    out = np.zeros((NCALL, 128, IDXW), np.int16)
    for j, (nidx, off) in enumerate(zip(CALLS, OFFS)):
        r = a[off:off + nidx].reshape(nidx // 16, 16).T  # [16, nidx/16]
        out[j, :, : nidx // 16] = np.tile(r, (8, 1))
    return out


def _host_plan(parents):
    """Per-core index plans. Returns (pidx, iidx, dest) where
    pidx/iidx: [N_CORES, N_WIN*NCALL, 128, IDXW] int16,
    dest: [N_CORES, SLOTS] int64 (-1 for discarded pad slots)."""
    parents = np.asarray(parents).astype(np.int64).ravel()
    pidx_all = np.zeros((N_CORES, N_WIN, SPW), np.int16)
    iidx_all = np.zeros((N_CORES, N_WIN, SPW), np.int16)
    dest_all = np.full((N_CORES, N_WIN, SPW), -1, np.int64)
    for c in range(N_CORES):
        o_loc = np.arange(RPC, dtype=np.int64)
        i_glob = c * RPC + o_loc + 1
        valid = i_glob < SEQ                 # final slot of core 7 is dummy
        p = np.where(valid, parents[np.minimum(i_glob, SEQ - 1)], 0)
        order = np.argsort(p, kind="stable")
        ps = p[order]
        w_id = ps // WIN
        counts = np.bincount(w_id, minlength=N_WIN)
        if counts.max() > SPW:
            raise ValueError(f"window overflow: {counts}")
        offs = np.concatenate(([0], np.cumsum(counts)))
        for w in range(N_WIN):
            k = counts[w]
            seg = slice(offs[w], offs[w] + k)
            pidx_all[c, w, :k] = (ps[seg] - w * WIN).astype(np.int16)
            olocs = order[seg]
            iidx_all[c, w, :k] = olocs.astype(np.int16)
            d = c * RPC + olocs
            dest_all[c, w, :k] = np.where(d + 1 < SEQ, d, -1)
    pidx_w = np.stack([
        np.concatenate([_wrap_idx(pidx_all[c, w]) for w in range(N_WIN)])
        for c in range(N_CORES)])
    iidx_w = np.stack([
        np.concatenate([_wrap_idx(iidx_all[c, w]) for w in range(N_WIN)])
        for c in range(N_CORES)])
    return pidx_w, iidx_w, dest_all.reshape(N_CORES, SLOTS)


def _make_in_maps(x_f32, parents, W1, b1, W2, b2):
    xbf = x_f32.astype(BF16)
    w1t = np.ascontiguousarray(np.asarray(W1, np.float32).T).astype(BF16)
    w2t = np.ascontiguousarray(np.asarray(W2, np.float32).T).astype(BF16)
    b1c = np.asarray(b1, np.float32).reshape(H2, 1).copy()
    b2r = np.ascontiguousarray(
        np.broadcast_to(np.asarray(b2, np.float32).reshape(1, OUT_DIM),
                        (128, OUT_DIM)))
    pidx_w, iidx_w, dest = _host_plan(parents)
    in_maps = []
    for c in range(N_CORES):
        lo = c * RPC + 1
        hi = min(lo + RPC, SEQ)
        xloc = np.zeros((RPC, H2), BF16)
        xloc[: hi - lo] = xbf[lo:hi]
        in_maps.append({
            "xbf": xbf, "xloc": xloc,
            "pidx": pidx_w[c], "iidx": iidx_w[c],
            "w1t": w1t, "w2t": w2t, "b1c": b1c, "b2r": b2r,
        })
    return in_maps, dest


def kernel(sent_encoding, parents, W1, b1, W2, b2):
    x = np.asarray(sent_encoding, dtype=np.float32).reshape(SEQ, H2)
    in_maps, dest = _make_in_maps(x, parents, W1, b1, W2, b2)
    nc = _get_program()
    res = run_bass_kernel_spmd(nc, in_maps, list(range(N_CORES)))

    out_full = np.empty((SEQ - 1, OUT_DIM), np.float32)
    for c in range(N_CORES):
        oc = np.asarray(res.results[c]["out"], np.float32)
        d = dest[c]
        m = d >= 0
        out_full[d[m]] = oc[m]
    return out_full


# revision 9
# speedup vs baseline: 1.6617x; 1.6617x over previous
"""Trainium2 Bass kernel for the DiscourseAct GNN message-passing problem.

Computation (reference):
    x   = sent_encoding[0]                       # [SEQ, 256] f32
    du  = concat(x[1:], x[parents[1:]], axis=1)  # [SEQ-1, 512]
    pre = tanh(du @ W1.T + b1)                   # [SEQ-1, 256]
    out = pre @ W2.T + b2                        # [SEQ-1, 43]

Strategy (8 cores, data-parallel over rows, locality-aware index
partitioning):
  * Output rows are sharded contiguously: core c owns rows
    [c*32768, (c+1)*32768) (the final slot of core 7 is a dummy).
  * Host-side preprocessing (index manipulation + shard layout only):
    each core's rows are ordered by parent index and split into 8 windows
    of 32768 source rows so the on-device `dma_gather` (int16 indices)
    can address the parents; each window segment is padded to a fixed
    4352 slots.  The core's OWN rows (`x[1:]` is a contiguous slice in
    the reference) are laid out in slot order, feature-major, so the
    device streams them with plain sequential DMA; only the actual
    message-passing gather `x[parents]` runs on device.
  * Device, per window: 5 bf16 transpose-mode dma_gathers (<=896 idxs per
    call, HW limit <1024) spread over 4 SWDGE queues fetch parent rows
    from the full replicated x directly into feature-major SBUF layout;
    one sequential DMA loads the own-row slab.  Then a bf16 matmul
    pipeline:
      L1: h.T[256, N] accumulated in PSUM over K=512,
      tanh+b1 on the scalar engine (PSUM -> SBUF, bf16),
      L2: out[128, 43] with h.T slices as the stationary operand,
      +b2 on the vector engine, contiguous DMA to DRAM.
  * Host reassembles (un-permutes) the rows into the full output.
"""

import numpy as np
import ml_dtypes

import concourse.bass as bass
import concourse.tile as tile
from concourse import bacc, mybir
from concourse.bass_utils import run_bass_kernel_spmd

BF16 = ml_dtypes.bfloat16

SEQ = 262144
H2 = 256            # feature dim of x
OUT_DIM = 43
N_CORES = 8
RPC = SEQ // N_CORES          # rows (output slots) per core = 32768
N_WIN = 8
WIN = SEQ // N_WIN            # rows per gather window = 32768
SPW = 4352                    # padded slots per window (34 * 128)
SLOTS = N_WIN * SPW           # 34816 compute slots per core
CALLS = (896, 896, 896, 896, 768)   # gather call sizes per window (sum=SPW)
OFFS = (0, 896, 1792, 2688, 3584)
NCALL = len(CALLS)
IDXW = CALLS[0] // 16         # idx tile cols (896/16 = 56)
CHUNK = 512                   # L1 moving-dim chunk (psum bank = 512 f32)
NQ = 4                        # SWDGE queues for gathers


def _build_program():
    nc = bacc.Bacc("TRN2", target_bir_lowering=False, debug=False,
                   num_devices=N_CORES, num_swdge_queues=NQ)
    dt = mybir.dt

    xbf_d = nc.dram_tensor("xbf", [SEQ, H2], dt.bfloat16, kind="ExternalInput")
    # own rows, slot order, feature-major: [w][feat%128][feat//128][slot]
    xin_d = nc.dram_tensor("xin", [N_WIN, 128, 2, SPW], dt.bfloat16,
                           kind="ExternalInput")
    pidx_d = nc.dram_tensor("pidx", [N_WIN * NCALL, 128, IDXW], dt.int16,
                            kind="ExternalInput")
    w1t_d = nc.dram_tensor("w1t", [2 * H2, H2], dt.bfloat16, kind="ExternalInput")
    w2t_d = nc.dram_tensor("w2t", [H2, OUT_DIM], dt.bfloat16, kind="ExternalInput")
    b1_d = nc.dram_tensor("b1c", [H2, 1], dt.float32, kind="ExternalInput")
    b2r_d = nc.dram_tensor("b2r", [128, OUT_DIM], dt.float32, kind="ExternalInput")
    out_d = nc.dram_tensor("out", [SLOTS, OUT_DIM], dt.float32,
                           kind="ExternalOutput")

    with tile.TileContext(nc) as tc:
        with (
            tc.tile_pool(name="const", bufs=1) as const,
            tc.tile_pool(name="gp_pool", bufs=6) as gpool,
            tc.tile_pool(name="xi_pool", bufs=2) as xpool,
            tc.tile_pool(name="ht", bufs=3) as hpool,
            tc.tile_pool(name="osb", bufs=4) as opool,
            tc.tile_pool(name="hps", bufs=2, space="PSUM") as psum_h,
            tc.tile_pool(name="ops", bufs=4, space="PSUM") as psum_o,
        ):
            # --- one-time loads -------------------------------------------
            w1t_sb = []
            for k in range(4):
                t = const.tile([128, H2], dt.bfloat16, tag=f"w1t{k}",
                               name=f"w1t_sb{k}")
                nc.sync.dma_start(t[:], w1t_d.ap()[k * 128:(k + 1) * 128, :])
                w1t_sb.append(t)
            w2t_sb = []
            for k in range(2):
                t = const.tile([128, OUT_DIM], dt.bfloat16, tag=f"w2t{k}",
                               name=f"w2t_sb{k}")
                nc.sync.dma_start(t[:], w2t_d.ap()[k * 128:(k + 1) * 128, :])
                w2t_sb.append(t)
            b1_sb = []
            for m in range(2):
                t = const.tile([128, 1], dt.float32, tag=f"b1{m}",
                               name=f"b1_sb{m}")
                nc.sync.dma_start(t[:], b1_d.ap()[m * 128:(m + 1) * 128, :])
                b1_sb.append(t)
            b2r_sb = const.tile([128, OUT_DIM], dt.float32, tag="b2r")
            nc.sync.dma_start(b2r_sb[:], b2r_d.ap())
            pidx_sb = []
            for ci in range(N_WIN * NCALL):
                tp = const.tile([128, IDXW], dt.int16, tag=f"pidx{ci}",
                                name=f"pidx_sb{ci}")
                nc.sync.dma_start(tp[:], pidx_d.ap()[ci])
                pidx_sb.append(tp)

            # --- main loop ------------------------------------------------
            for w in range(N_WIN):
                gi = xpool.tile([128, 2, SPW], dt.bfloat16, tag="gi",
                                name=f"gi_{w}")
                nc.sync.dma_start(gi[:], xin_d.ap()[w])
                for j, (nidx, off) in enumerate(zip(CALLS, OFFS)):
                    ci = w * NCALL + j
                    iw = nidx // 16
                    gp = gpool.tile([128, 2, nidx], dt.bfloat16,
                                    tag=f"gp{nidx}", name=f"gp_{ci}")
                    nc.gpsimd.dma_gather(
                        gp[:], xbf_d.ap()[w * WIN:(w + 1) * WIN, :],
                        pidx_sb[ci][:, 0:iw],
                        num_idxs=nidx, num_idxs_reg=nidx, elem_size=H2,
                        transpose=True, queue_num=0,
                    )
                    for s0 in range(0, nidx, CHUNK):
                        L = min(CHUNK, nidx - s0)
                        c0 = off + s0      # column offset in gi slab
                        hps = [psum_h.tile([128, L], dt.float32, tag=f"h{m}",
                                           name=f"hps{m}")
                               for m in range(2)]
                        for m in range(2):
                            msl = slice(m * 128, (m + 1) * 128)
                            nc.tensor.matmul(hps[m][:], lhsT=w1t_sb[0][:, msl],
                                             rhs=gi[:, 0, c0:c0 + L],
                                             start=True, stop=False)
                            nc.tensor.matmul(hps[m][:], lhsT=w1t_sb[1][:, msl],
                                             rhs=gi[:, 1, c0:c0 + L],
                                             start=False, stop=False)
                            nc.tensor.matmul(hps[m][:], lhsT=w1t_sb[2][:, msl],
                                             rhs=gp[:, 0, s0:s0 + L],
                                             start=False, stop=False)
                            nc.tensor.matmul(hps[m][:], lhsT=w1t_sb[3][:, msl],
                                             rhs=gp[:, 1, s0:s0 + L],
                                             start=False, stop=True)
                        hts = []
                        for m in range(2):
                            ht = hpool.tile([128, L], dt.bfloat16, tag=f"ht{m}",
                                            name=f"ht{m}")
                            nc.scalar.activation(
                                ht[:], hps[m][:],
                                mybir.ActivationFunctionType.Tanh,
                                bias=b1_sb[m][:], scale=1.0)
                            hts.append(ht)
                        for r in range(0, L, 128):
                            ops = psum_o.tile([128, OUT_DIM], dt.float32,
                                              tag="o", name="ops")
                            nc.tensor.matmul(ops[:], lhsT=hts[0][:, r:r + 128],
                                             rhs=w2t_sb[0][:],
                                             start=True, stop=False)
                            nc.tensor.matmul(ops[:], lhsT=hts[1][:, r:r + 128],
                                             rhs=w2t_sb[1][:],
                                             start=False, stop=True)
                            osb = opool.tile([128, OUT_DIM], dt.float32,
                                             tag="osb", name="osb")
                            nc.vector.tensor_add(osb[:], ops[:], b2r_sb[:])
                            row0 = w * SPW + off + s0 + r
                            nc.sync.dma_start(out_d.ap()[row0:row0 + 128, :],
                                              osb[:])

    # Tile assigns each Pool-DMA a DMASW sem lane round-robin in scheduled
    # order; the ucode locks each lane's sem to one SWDGE queue. Spread the
    # gathers over the 4 queues by deriving queue_num from the assigned lane.
    from concourse.tile_sem_assignment import PROC_NAME_TO_IDX
    dmasw0 = PROC_NAME_TO_IDX["DMASW0"]
    for func in nc.m.functions:
        for block in func.blocks:
            for inst in block.instructions:
                if isinstance(inst, mybir.InstDMAGatherAnt):
                    proc = inst.bass_scheduled_proc
                    assert proc is not None and dmasw0 <= proc < dmasw0 + 8, proc
                    inst.queue_num = (proc - dmasw0) % NQ

    nc.compile()
    return nc


_CACHE = {}


def _get_program():
    if "nc" not in _CACHE:
        _CACHE["nc"] = _build_program()
    return _CACHE["nc"]


def _wrap_idx(a):
    """[SPW] int16 -> [NCALL, 128, IDXW] per-call wrapped layout:
    out[j, p, s] = call_j[s*16 + (p % 16)], zero-padded to IDXW cols."""
    out = np.zeros((NCALL, 128, IDXW), np.int16)
    for j, (nidx, off) in enumerate(zip(CALLS, OFFS)):
        r = a[off:off + nidx].reshape(nidx // 16, 16).T  # [16, nidx/16]
        out[j, :, : nidx // 16] = np.tile(r, (8, 1))
    return out


def _host_plan(parents):
    """Per-core index plans. Returns (pidx, order, dest) where
    pidx: [N_CORES, N_WIN*NCALL, 128, IDXW] int16,
    order: [N_CORES, N_WIN, SPW] int64 local row index per slot (-1 = pad),
    dest: [N_CORES, SLOTS] int64 (-1 for discarded pad slots)."""
    parents = np.asarray(parents).astype(np.int64).ravel()
    pidx_all = np.zeros((N_CORES, N_WIN, SPW), np.int16)
    order_all = np.full((N_CORES, N_WIN, SPW), -1, np.int64)
    dest_all = np.full((N_CORES, N_WIN, SPW), -1, np.int64)
    for c in range(N_CORES):
        o_loc = np.arange(RPC, dtype=np.int64)
        i_glob = c * RPC + o_loc + 1
        valid = i_glob < SEQ                 # final slot of core 7 is dummy
        p = np.where(valid, parents[np.minimum(i_glob, SEQ - 1)], 0)
        order = np.argsort(p, kind="stable")
        ps = p[order]
        w_id = ps // WIN
        counts = np.bincount(w_id, minlength=N_WIN)
        if counts.max() > SPW:
            raise ValueError(f"window overflow: {counts}")
        offs = np.concatenate(([0], np.cumsum(counts)))
        for w in range(N_WIN):
            k = counts[w]
            seg = slice(offs[w], offs[w] + k)
            pidx_all[c, w, :k] = (ps[seg] - w * WIN).astype(np.int16)
            olocs = order[seg]
            order_all[c, w, :k] = olocs
            d = c * RPC + olocs
            dest_all[c, w, :k] = np.where(d + 1 < SEQ, d, -1)
    pidx_w = np.stack([
        np.concatenate([_wrap_idx(pidx_all[c, w]) for w in range(N_WIN)])
        for c in range(N_CORES)])
    return pidx_w, order_all, dest_all.reshape(N_CORES, SLOTS)


def _make_in_maps(x_f32, parents, W1, b1, W2, b2):
    xbf = x_f32.astype(BF16)
    w1t = np.ascontiguousarray(np.asarray(W1, np.float32).T).astype(BF16)
    w2t = np.ascontiguousarray(np.asarray(W2, np.float32).T).astype(BF16)
    b1c = np.asarray(b1, np.float32).reshape(H2, 1).copy()
    b2r = np.ascontiguousarray(
        np.broadcast_to(np.asarray(b2, np.float32).reshape(1, OUT_DIM),
                        (128, OUT_DIM)))
    pidx_w, order_all, dest = _host_plan(parents)
    in_maps = []
    for c in range(N_CORES):
        lo = c * RPC + 1
        hi = min(lo + RPC, SEQ)
        xloc = np.zeros((RPC, H2), BF16)       # row o_loc = x[c*RPC+1+o_loc]
        xloc[: hi - lo] = xbf[lo:hi]
        # own rows in slot order, feature-major per window
        xin = np.zeros((N_WIN, 128, 2, SPW), BF16)
        for w in range(N_WIN):
            sel = order_all[c, w]
            rows = xloc[np.maximum(sel, 0)]          # [SPW, 256]
            rows[sel < 0] = 0
            # [SPW, 256] -> [256, SPW] -> [2, 128, SPW] -> [128, 2, SPW]
            xin[w] = rows.T.reshape(2, 128, SPW).transpose(1, 0, 2)
        in_maps.append({
            "xbf": xbf, "xin": xin, "pidx": pidx_w[c],
            "w1t": w1t, "w2t": w2t, "b1c": b1c, "b2r": b2r,
        })
    return in_maps, dest


def kernel(sent_encoding, parents, W1, b1, W2, b2):
    x = np.asarray(sent_encoding, dtype=np.float32).reshape(SEQ, H2)
    in_maps, dest = _make_in_maps(x, parents, W1, b1, W2, b2)
    nc = _get_program()
    res = run_bass_kernel_spmd(nc, in_maps, list(range(N_CORES)))

    out_full = np.empty((SEQ - 1, OUT_DIM), np.float32)
    for c in range(N_CORES):
        oc = np.asarray(res.results[c]["out"], np.float32)
        d = dest[c]
        m = d >= 0
        out_full[d[m]] = oc[m]
    return out_full
